# revision 18
# baseline (speedup 1.0000x reference)
"""Trainium2 Bass kernel for PoolingPMATopK.

Reference computation (per batch b, query q):
  scores[q, n] = seed[q] . x[b, n]          (n = 0..8191, h = 768)
  top-128 of scores -> softmax(vals * 12^-0.5) -> weighted sum of x rows.

Strategy per core (2 batches, batch-data-parallel over 8 cores):
  - Host pre-casts x to fp16 (identical values to an on-chip cast) and
    pre-transposes h-blocks 0-2; HBM read per 512-row window is 1.18MB
    (natural fp16 + 3 transposed blocks), balancing DMA (~2.9us/window)
    against PE (12 fp16 block transposes for h-blocks 3-5 + mm1).
  - mm1 fp16: h-blocks 0-2 as three N=512 matmuls right after the
    transposed DMA; blocks 3-5 as chunk-pair N=256 matmuls that
    pipeline behind the ACT copies.  The 32-wide qT stationary is
    placed via tile_position=(0, 32*(w%4)) so PSUM output lands
    directly on scores partitions 32*(w%4).  Scores stay fp32.
  - Group postprocessing (strip copy, E=exp(c*s) fp16, L1 top-24) is
    deferred one window so the ACT queue never stalls on mm1.
    exp needs no max subtraction (softmax ratio is shift invariant,
    |c*s| < 2).
  - Exact theta via staged merges, mostly hidden under the stream:
    P1 = top128(G0 u G1), P2 = top128(P1 u G2), then an exposed
    bottom-rank extraction: theta is the (c'+1)-th smallest of
    P2 u {G3 candidates >= min(P2)} where c' = |{G3 >= min(P2)}|
    (c' <= 65 on this distribution; 10 min8 rounds cover c' <= 79).
  - w16 = 1[s >= theta]*E (fp16); phase B transposes w16 at fp16 cost;
    mm2 fp16 col-tiled with a ones-column per chunk giving Z in the
    same matmul.  out = (w @ x) / Z.
  - Window loads ride the sync queue exclusively; small tail DMAs
    (candidate gathers, theta broadcast, output) ride the scalar
    queue so batch 1's stream is never head-of-line blocked.
  - 32-chunk overlay residency lets the odd batch stream while the
    even batch's mm2 still reads the resident tile.
"""

import numpy as np

B, N, H, Q = 16, 8192, 768, 32
NCORES = 8
BPC = B // NCORES          # batches per core
NCH = N // 128             # 64 chunks of 128 rows per batch
KB = H // 128              # 6 h-blocks
KT = 3                     # h-blocks host-transposed (DMA'd directly)
WPB = N // 512             # 16 windows per batch
CW = H + 1                 # 769 resident cols per chunk (ones + data)
CSCALE = float(12 ** -0.5)
WTOP = 24                  # candidates kept per 512-col group (true max 19)
MR = 10                    # min8 rounds in the final merge (covers c'<=79)
OVER = 32                  # chunks of overlay residency for odd batches
NEG = -1e30
BIG = 1e30
THETA_SIMPLE = True   # bisect flag: plain 16-round final L2 over 384 cands
DEBUG_DUMP = False    # dump scores + theta to a debug DRAM tensor

_built = None


def _apply_patches():
    """Inline of tile_patch.py: the TileContext final Drain carries one wait
    per pending semaphore lane (walrus allows at most 1 sync wait per
    instruction on TRN2)."""
    import bass_rust as _br
    from concourse import tile as _tile
    from concourse.tile_scheduler import N_PROCS

    def _patched_drain_and_barrier(self, tick_clock, wait_clock):
        sems = self.sems.allocated()
        gc = tick_clock.global_clock
        for p in range(N_PROCS):
            tick = gc[p]
            if tick <= 0:
                continue
            sem = sems.get(p)
            if sem is None:
                continue
            self.nc.sync.wait_ge(sem, _br.tick_to_sem(tick, p))
        self.nc.sync.drain()
        self.nc.all_engine_barrier()
        assert self.sems is not None
        popped = self.nc._tile_sem_poison_stack.pop()
        assert popped is self._sem_poison
        self.nc.clear_and_free_semaphores(list(self.sems.allocated().values()))
        self.nc.all_engine_barrier()

    _tile.TileContext._drain_and_barrier = _patched_drain_and_barrier


def _build():
    import concourse.bass as bass
    import concourse.tile as tile
    from concourse import mybir

    _apply_patches()

    F32 = mybir.dt.float32
    F16 = mybir.dt.float16
    I32 = mybir.dt.int32
    COPY = mybir.ActivationFunctionType.Copy
    EXP = mybir.ActivationFunctionType.Exp
    ALU = mybir.AluOpType
    AXX = mybir.AxisListType.X

    nc = bass.Bass()
    x_d = nc.declare_dram_parameter("x16", [BPC * N, H], F16, isOutput=False)
    xt_d = nc.declare_dram_parameter(
        "x16t", [KT * 128, BPC * N], F16, isOutput=False
    )
    qT_d = nc.declare_dram_parameter("seedT", [H, Q], F32, isOutput=False)
    id_d = nc.declare_dram_parameter("ident", [128, 128], F32, isOutput=False)
    out_d = nc.declare_dram_parameter("out", [BPC * Q, H], F32, isOutput=True)
    if DEBUG_DUMP:
        dbg_d = nc.declare_dram_parameter(
            "dbg", [BPC * 128, 2048 + 1 + 4 * WTOP], F32, isOutput=True
        )

    with tile.TileContext(nc) as tc:
        with (
            tc.tile_pool(name="const", bufs=1) as cpool,
            tc.tile_pool(name="xt", bufs=2) as xtpool,
            tc.tile_pool(name="sc", bufs=2) as scpool,
            tc.tile_pool(name="work", bufs=1) as wpool,
            tc.tile_pool(name="ps_tp", bufs=2, space="PSUM") as ps_tp,
            tc.tile_pool(name="ps_m", bufs=2, space="PSUM") as ps_m,
            tc.tile_pool(name="ps_b", bufs=2, space="PSUM") as ps_b,
            tc.tile_pool(name="ps_2", bufs=1, space="PSUM") as ps_2,
        ):
            res_t = wpool.tile([128, NCH * CW], F16)
            nc.vector.memset(res_t[:, 0:NCH * CW:CW], 1.0)
            res2_t = wpool.tile([128, OVER * CW], F16)
            nc.vector.memset(res2_t[:, 0:OVER * CW:CW], 1.0)

            id_t = cpool.tile([128, 128], F32)
            nc.scalar.dma_start(id_t[:], id_d[:])
            id16_t = cpool.tile([128, 128], F16)
            nc.vector.tensor_copy(id16_t[:], id_t[:])

            qT_f32 = cpool.tile([128, KB * 32], F32)
            for k in range(KB):
                nc.scalar.dma_start(
                    qT_f32[:, 32 * k:32 * k + 32], qT_d[128 * k:128 * k + 128, :]
                )
            qT_t = cpool.tile([128, KB * 32], F16)
            nc.vector.tensor_copy(qT_t[:], qT_f32[:])

            iota_i = cpool.tile([32, 8 * MR], I32)
            nc.gpsimd.iota(iota_i[:], pattern=[[1, 8 * MR]], base=0,
                           channel_multiplier=0)
            iota_f = cpool.tile([32, 8 * MR], F32)
            nc.vector.tensor_copy(iota_f[:], iota_i[:])

            scratch_t = wpool.tile([128, 512], F32)
            cand_t = wpool.tile([128, 4 * WTOP], F32)
            p1src_t = wpool.tile([32, 8 * WTOP], F32)    # G0|G1 cands (192)
            p2src_t = wpool.tile([32, 128 + 4 * WTOP], F32)  # P1 | G2 (224)
            pfin_t = wpool.tile([32, 128], F32)          # P1 then P2
            g3_t = wpool.tile([32, 4 * WTOP], F32)
            c2_t = wpool.tile([32, 16 * WTOP], F32)
            tt_t = wpool.tile([32, 128], F32)
            ge_t = wpool.tile([32, 4 * WTOP], F32)
            mrg_t = wpool.tile([32, 80 + 4 * WTOP], F32)  # -P2[48:] | -G3f
            minb_t = wpool.tile([32, 8 * MR], F32)
            cnt_t = wpool.tile([32, 1], F32)
            eq_t = wpool.tile([32, 8 * MR], F32)
            thn_t = wpool.tile([32, 1], F32)
            thr4_t = wpool.tile([32, 4], F32)
            b_th = wpool.tile([128, 1], F32)
            rz_t = wpool.tile([32, 1], F32)
            o2_t = wpool.tile([32, H], F32)
            w16_t = wpool.tile([128, 2048], F16)
            wT_sb = [
                wpool.tile([128, 512], F16, name=f"wT_sb{u}") for u in range(4)
            ]
            sel_t = wpool.tile([128, 32], F32)
            nc.vector.memset(sel_t[:], 0.0)
            for g in range(4):
                nc.vector.tensor_copy(
                    sel_t[32 * g:32 * g + 32, :],
                    id_t[32 * g:32 * g + 32, 32 * g:32 * g + 32],
                )
            sba_t = wpool.tile([128, 385], F32)
            sbb_t = wpool.tile([128, 384], F32)

            cand4 = cand_t.rearrange("(j p) c -> p j c", j=4)

            def res_chunk(bb, c):
                """Residency slice [128, CW] for chunk c of batch bb."""
                if bb % 2 == 1 and c < OVER:
                    return res2_t[:, CW * c:CW * c + CW]
                return res_t[:, CW * c:CW * c + CW]

            def res_win(bb, w):
                """Residency slice [128, 4*CW] for window w of batch bb
                (windows never straddle the res/res2 boundary)."""
                c0 = 4 * w
                if bb % 2 == 1 and c0 < OVER:
                    return res2_t[:, CW * c0:CW * (c0 + 4)]
                return res_t[:, CW * c0:CW * (c0 + 4)]

            for b in range(BPC):
                row0 = b * N
                sc_t = scpool.tile([128, 2048], F32, name="scores")
                e16_t = scpool.tile([128, 2048], F16, name="expsc")

                def postproc(t, pw):
                    """Strip copy + exp + L1 top-24 for column group t,
                    then the hidden merge stage it unlocks."""
                    cs = slice(512 * t, 512 * t + 512)
                    nc.scalar.activation(sc_t[:, cs], pw[:], COPY)
                    nc.scalar.activation(
                        e16_t[:, cs], pw[:], EXP, scale=CSCALE,
                    )
                    cnd = cand_t[:, WTOP * t:WTOP * t + WTOP]
                    nc.vector.max(cnd[:, 0:8], sc_t[:, cs])
                    nc.vector.match_replace(
                        scratch_t[:], cnd[:, 0:8], sc_t[:, cs], NEG
                    )
                    nc.vector.max(cnd[:, 8:16], scratch_t[:])
                    nc.vector.match_replace(
                        scratch_t[:], cnd[:, 8:16], scratch_t[:], NEG
                    )
                    nc.vector.max(cnd[:, 16:24], scratch_t[:])
                    if t == 1:
                        # P1 = top-128 of G0|G1's 192 candidates (hidden)
                        for jj in range(4):
                            nc.scalar.dma_start(
                                p1src_t[:, 48 * jj:48 * jj + 48],
                                cand_t[32 * jj:32 * jj + 32, 0:2 * WTOP],
                            )
                        for r in range(16):
                            nc.vector.max(
                                pfin_t[:, 8 * r:8 * r + 8], p1src_t[:]
                            )
                            if r < 15:
                                nc.vector.match_replace(
                                    p1src_t[:], pfin_t[:, 8 * r:8 * r + 8],
                                    p1src_t[:], NEG,
                                )
                    elif t == 2:
                        # P2 = top-128 of P1 | G2's 96 cands (hidden)
                        nc.vector.tensor_copy(p2src_t[:, 0:128], pfin_t[:])
                        for jj in range(4):
                            nc.scalar.dma_start(
                                p2src_t[:, 128 + WTOP * jj:
                                        128 + WTOP * jj + WTOP],
                                cand_t[32 * jj:32 * jj + 32,
                                       2 * WTOP:3 * WTOP],
                            )
                        for r in range(16):
                            nc.vector.max(
                                pfin_t[:, 8 * r:8 * r + 8], p2src_t[:]
                            )
                            if r < 15:
                                nc.vector.match_replace(
                                    p2src_t[:], pfin_t[:, 8 * r:8 * r + 8],
                                    p2src_t[:], NEG,
                                )

                # ---- Phase A: stream windows.
                pending = None
                pw = None
                for w in range(WPB):
                    jq = w % 4          # partition strip
                    nc.sync.dma_start(
                        res_win(b, w).rearrange(
                            "p (c e) -> p c e", c=4
                        )[:, :, 1:1 + H],
                        x_d[row0 + 512 * w:row0 + 512 * w + 512, :].rearrange(
                            "(c p) h -> p c h", p=128
                        ),
                    )
                    xt = xtpool.tile([128, KB * 512], F16)
                    xt3 = xt.rearrange("p (k i) -> p k i", k=KB)
                    nc.sync.dma_start(
                        xt3[:, 0:KT, :],
                        xt_d[:, row0 + 512 * w:row0 + 512 * w + 512].rearrange(
                            "(k p) n -> p k n", p=128
                        ),
                    )
                    if jq == 0:
                        pw = ps_m.tile([128, 512], F32, name="pw")
                    # mm1a: h-blocks 0..KT-1, N=512, straight off the DMA
                    for k in range(KT):
                        nc.tensor.matmul(
                            pw[32 * jq:32 * jq + 32, :],
                            qT_t[:, 32 * k:32 * k + 32],
                            xt[:, 512 * k:512 * k + 512],
                            start=(k == 0), stop=False,
                            skip_group_check=True,
                            tile_position=(0, 32 * jq),
                        )
                    for cw in range(4):
                        c = 4 * w + cw
                        src = res_chunk(b, c)[:, 1:1 + H]
                        # transpose h-blocks KT..5 of the fp16 chunk
                        tp = ps_tp.tile([128, KB - KT, 128], F16, name="tp")
                        for k in range(KT, KB):
                            nc.tensor.matmul(
                                tp[:, k - KT, :],
                                src[:, 128 * k:128 * k + 128],
                                id16_t[:],
                                is_transpose=True, start=True, stop=True,
                                skip_group_check=True,
                            )
                        dst = xt3[:, KT:KB, 128 * cw:128 * cw + 128]
                        nc.scalar.activation(dst, tp[:], COPY)
                        if cw % 2 == 1:
                            # mm1b: blocks KT..5 for this chunk pair, N=256
                            pcs = slice(128 * (cw - 1), 128 * (cw - 1) + 256)
                            for k in range(KT, KB):
                                nc.tensor.matmul(
                                    pw[32 * jq:32 * jq + 32, pcs],
                                    qT_t[:, 32 * k:32 * k + 32],
                                    xt[:, 512 * k + 128 * (cw - 1):
                                       512 * k + 128 * (cw - 1) + 256],
                                    start=False,
                                    stop=(k == KB - 1),
                                    skip_group_check=True,
                                    tile_position=(0, 32 * jq),
                                )
                    if pending is not None:
                        postproc(*pending)
                        pending = None
                    if jq == 3:
                        pending = (w // 4, pw)
                postproc(*pending)
                pending = None

                # ---- Phase A2 (exposed): theta = 128th of P2 | G3f via
                #      bottom-rank extraction.
                if THETA_SIMPLE:
                    for jj in range(4):
                        nc.scalar.dma_start(
                            c2_t[:, 96 * jj:96 * jj + 96],
                            cand_t[32 * jj:32 * jj + 32, :],
                        )
                    for r in range(16):
                        nc.vector.max(tt_t[:, 8 * r:8 * r + 8], c2_t[:])
                        if r < 15:
                            nc.vector.match_replace(
                                c2_t[:], tt_t[:, 8 * r:8 * r + 8],
                                c2_t[:], NEG,
                            )
                    for jj in range(4):
                        nc.scalar.dma_start(
                            b_th[32 * jj:32 * jj + 32, :], tt_t[:, 127:128]
                        )
                else:
                    for jj in range(4):
                        nc.scalar.dma_start(
                            g3_t[:, WTOP * jj:WTOP * jj + WTOP],
                            cand_t[32 * jj:32 * jj + 32, 3 * WTOP:4 * WTOP],
                        )
                    t0 = pfin_t[:, 127:128]
                    # ge = 1[g3 >= t0]; c' = row-sum(ge)
                    nc.vector.tensor_scalar(
                        ge_t[:], g3_t[:], t0, None, ALU.is_ge
                    )
                    nc.vector.tensor_reduce(cnt_t[:], ge_t[:], AXX, ALU.add)
                    # mrg = [-P2[48:128] | (-G3 masked to -BIG below t0)]
                    nc.vector.tensor_scalar_mul(
                        mrg_t[:, 0:80], pfin_t[:, 48:128], -1.0
                    )
                    nc.vector.tensor_scalar(
                        mrg_t[:, 80:176], g3_t[:], t0, -BIG, ALU.is_lt,
                        ALU.mult
                    )
                    nc.vector.tensor_tensor(
                        mrg_t[:, 80:176], mrg_t[:, 80:176], g3_t[:],
                        ALU.subtract
                    )
                    for r in range(MR):
                        nc.vector.max(minb_t[:, 8 * r:8 * r + 8], mrg_t[:])
                        if r < MR - 1:
                            nc.vector.match_replace(
                                mrg_t[:], minb_t[:, 8 * r:8 * r + 8],
                                mrg_t[:], NEG,
                            )
                    # theta = -minb[c']
                    nc.vector.tensor_scalar(
                        eq_t[:], iota_f[:], cnt_t[:, 0:1], None, ALU.is_equal
                    )
                    nc.vector.tensor_tensor(
                        eq_t[:], eq_t[:], minb_t[:], ALU.mult
                    )
                    nc.vector.tensor_reduce(thn_t[:], eq_t[:], AXX, ALU.add)
                    nc.vector.tensor_scalar_mul(thr4_t[:, 0:1], thn_t[:], -1.0)
                    for jj in range(4):
                        nc.scalar.dma_start(
                            b_th[32 * jj:32 * jj + 32, :], thr4_t[:, 0:1]
                        )

                if DEBUG_DUMP:
                    nc.scalar.dma_start(
                        dbg_d[128 * b:128 * b + 128, 0:2048], sc_t[:]
                    )
                    nc.scalar.dma_start(
                        dbg_d[128 * b:128 * b + 128, 2048:2049], b_th[:]
                    )
                    nc.scalar.dma_start(
                        dbg_d[128 * b:128 * b + 32, 2049:2049 + 4 * WTOP],
                        cand_t[0:32, :],
                    )

                # ---- Phase B interleaved with stt: per quarter u,
                #      w16 = 1[s>=theta]*E (fp16), then 4 fp16
                #      transposes of the quarter -> wT_sb[u].
                for u in range(4):
                    us = slice(512 * u, 512 * u + 512)
                    nc.vector.scalar_tensor_tensor(
                        w16_t[:, us], sc_t[:, us], b_th[:, 0:1],
                        e16_t[:, us],
                        ALU.is_ge, ALU.mult,
                    )
                    wtp = ps_b.tile([128, 512], F16, name="wtp")
                    for tt in range(4):
                        nc.tensor.matmul(
                            wtp[:, 128 * tt:128 * tt + 128],
                            w16_t[:, 512 * u + 128 * tt:
                                  512 * u + 128 * tt + 128],
                            id16_t[:],
                            is_transpose=True, start=True, stop=True,
                            skip_group_check=True,
                        )
                    nc.scalar.activation(wT_sb[u][:], wtp[:], COPY)

                # ---- Phase C: mm2 out = (w @ x) / Z, Z from ones column
                p2a = ps_2.tile([128, 385], F32)
                p2b = ps_2.tile([128, 384], F32)
                for i in range(NCH):
                    u = i // 16
                    tt = i % 4
                    jq = (i // 4) % 4
                    g = i % 4   # concurrent col-group (partition strip)
                    lhs = wT_sb[u][:, 128 * tt + 32 * jq:128 * tt + 32 * jq + 32]
                    src = res_chunk(b, i)
                    nc.tensor.matmul(
                        p2a[32 * g:32 * g + 32, :], lhs, src[:, 0:385],
                        start=(i < 4), stop=(i >= NCH - 4),
                        skip_group_check=True,
                        tile_position=(0, 32 * g),
                    )
                    nc.tensor.matmul(
                        p2b[32 * g:32 * g + 32, :], lhs, src[:, 385:CW],
                        start=(i < 4), stop=(i >= NCH - 4),
                        skip_group_check=True,
                        tile_position=(0, 32 * g),
                    )
                # merge the 4 strip partials: out[q] = sum_g partial[32g+q]
                nc.scalar.activation(sba_t[:], p2a[:], COPY)
                nc.scalar.activation(sbb_t[:], p2b[:], COPY)
                nc.tensor.matmul(
                    p2a[0:32, :], sel_t[:], sba_t[:],
                    start=True, stop=True, skip_group_check=True,
                )
                nc.tensor.matmul(
                    p2b[0:32, :], sel_t[:], sbb_t[:],
                    start=True, stop=True, skip_group_check=True,
                )
                nc.vector.reciprocal(rz_t[:], p2a[0:32, 0:1])
                nc.scalar.activation(
                    o2_t[:, 0:384], p2a[0:32, 1:385], COPY, scale=rz_t[:]
                )
                nc.scalar.activation(
                    o2_t[:, 384:768], p2b[0:32, :], COPY, scale=rz_t[:]
                )
                nc.scalar.dma_start(out_d[Q * b:Q * b + Q, :], o2_t[:])

    # Split multi-wait instructions to the TRN2 1-wait-per-instruction limit
    # (the standard Bacc.compile() passes, skipped on the bass2jax run path).
    import bass_rust as _bass_rust
    _bass_rust.move_matmul_waits_to_ldweights(nc.m)
    _bass_rust.generate_event_semaphores(nc)
    return nc


def _get_nc():
    global _built
    if _built is None:
        _built = _build()
    return _built


def run(inputs, trace=False):
    from concourse.bass_utils import run_bass_kernel_spmd

    x = np.asarray(inputs["input"], dtype=np.float32)
    seed = np.ascontiguousarray(np.asarray(inputs["seed"], dtype=np.float32))
    nc = _get_nc()
    seedT = np.ascontiguousarray(seed[0].T)
    ident = np.eye(128, dtype=np.float32)
    x16 = x.astype(np.float16).reshape(B * N, H)
    in_maps = []
    for c in range(NCORES):
        xb = np.ascontiguousarray(x16[BPC * N * c:BPC * N * (c + 1)])
        xbt = np.ascontiguousarray(xb[:, 0:KT * 128].T)
        in_maps.append(
            {"x16": xb, "x16t": xbt, "seedT": seedT, "ident": ident}
        )
    res = run_bass_kernel_spmd(nc, in_maps, list(range(NCORES)), trace=trace)
    out = np.empty((B, Q, H), np.float32)
    for c in range(NCORES):
        out[BPC * c:BPC * (c + 1)] = res.results[c]["out"].reshape(BPC, Q, H)
    return out, res


def kernel(**inputs):
    out, _ = run(inputs)
    return out


# revision 19
# speedup vs baseline: 1.0197x; 1.0197x over previous
"""Trainium2 Bass kernel for PoolingPMATopK.

Reference computation (per batch b, query q):
  scores[q, n] = seed[q] . x[b, n]          (n = 0..8191, h = 768)
  top-128 of scores -> softmax(vals * 12^-0.5) -> weighted sum of x rows.

Strategy per core (2 batches, batch-data-parallel over 8 cores):
  - Host pre-casts x to fp16 (identical values to an on-chip cast) and
    pre-transposes h-blocks 0-2; HBM read per 512-row window is 1.18MB
    (natural fp16 + 3 transposed blocks), balancing DMA (~2.9us/window)
    against PE (12 fp16 block transposes for h-blocks 3-5 + mm1).
  - mm1 fp16: h-blocks 0-2 as three N=512 matmuls right after the
    transposed DMA; blocks 3-5 as chunk-pair N=256 matmuls that
    pipeline behind the ACT copies.  The 32-wide qT stationary is
    placed via tile_position=(0, 32*(w%4)) so PSUM output lands
    directly on scores partitions 32*(w%4).  Scores stay fp32.
  - Group postprocessing (strip copy, E=exp(c*s) fp16, L1 top-24) is
    deferred one window so the ACT queue never stalls on mm1.
    exp needs no max subtraction (softmax ratio is shift invariant,
    |c*s| < 2).
  - Exact theta via staged merges, mostly hidden under the stream:
    P1 = top128(G0 u G1), P2 = top128(P1 u G2), then an exposed
    bottom-rank extraction: theta is the (c'+1)-th smallest of
    P2 u {G3 candidates >= min(P2)} where c' = |{G3 >= min(P2)}|
    (c' <= 65 on this distribution; 10 min8 rounds cover c' <= 79).
  - w16 = 1[s >= theta]*E (fp16); phase B transposes w16 at fp16 cost;
    mm2 fp16 col-tiled with a ones-column per chunk giving Z in the
    same matmul.  out = (w @ x) / Z.
  - Window loads ride the sync queue exclusively; small tail DMAs
    (candidate gathers, theta broadcast, output) ride the scalar
    queue so batch 1's stream is never head-of-line blocked.
  - 32-chunk overlay residency lets the odd batch stream while the
    even batch's mm2 still reads the resident tile.
"""

import numpy as np

B, N, H, Q = 16, 8192, 768, 32
NCORES = 8
BPC = B // NCORES          # batches per core
NCH = N // 128             # 64 chunks of 128 rows per batch
KB = H // 128              # 6 h-blocks
KT = 3                     # h-blocks host-transposed (DMA'd directly)
WPB = N // 512             # 16 windows per batch
CW = H + 1                 # 769 resident cols per chunk (ones + data)
CSCALE = float(12 ** -0.5)
WTOP = 24                  # candidates kept per 512-col group (true max 19)
MR = 10                    # min8 rounds in the final merge (covers c'<=79)
OVER = 32                  # chunks of overlay residency for odd batches
NEG = -1e30
BIG = 1e30
THETA_SIMPLE = True   # bisect flag: plain 16-round final L2 over 384 cands
DEBUG_DUMP = False    # dump scores + theta to a debug DRAM tensor

_built = None


def _apply_patches():
    """Inline of tile_patch.py: the TileContext final Drain carries one wait
    per pending semaphore lane (walrus allows at most 1 sync wait per
    instruction on TRN2)."""
    import bass_rust as _br
    from concourse import tile as _tile
    from concourse.tile_scheduler import N_PROCS

    def _patched_drain_and_barrier(self, tick_clock, wait_clock):
        sems = self.sems.allocated()
        gc = tick_clock.global_clock
        for p in range(N_PROCS):
            tick = gc[p]
            if tick <= 0:
                continue
            sem = sems.get(p)
            if sem is None:
                continue
            self.nc.sync.wait_ge(sem, _br.tick_to_sem(tick, p))
        self.nc.sync.drain()
        self.nc.all_engine_barrier()
        assert self.sems is not None
        popped = self.nc._tile_sem_poison_stack.pop()
        assert popped is self._sem_poison
        self.nc.clear_and_free_semaphores(list(self.sems.allocated().values()))
        self.nc.all_engine_barrier()

    _tile.TileContext._drain_and_barrier = _patched_drain_and_barrier


def _build():
    import concourse.bass as bass
    import concourse.tile as tile
    from concourse import mybir

    _apply_patches()

    F32 = mybir.dt.float32
    F16 = mybir.dt.float16
    I32 = mybir.dt.int32
    COPY = mybir.ActivationFunctionType.Copy
    EXP = mybir.ActivationFunctionType.Exp
    ALU = mybir.AluOpType
    AXX = mybir.AxisListType.X

    nc = bass.Bass()
    x_d = nc.declare_dram_parameter("x16", [BPC * N, H], F16, isOutput=False)
    xt_d = nc.declare_dram_parameter(
        "x16t", [KT * 128, BPC * N], F16, isOutput=False
    )
    qT_d = nc.declare_dram_parameter("seedT", [H, Q], F32, isOutput=False)
    id_d = nc.declare_dram_parameter("ident", [128, 128], F32, isOutput=False)
    out_d = nc.declare_dram_parameter("out", [BPC * Q, H], F32, isOutput=True)
    if DEBUG_DUMP:
        dbg_d = nc.declare_dram_parameter(
            "dbg", [BPC * 128, 2048 + 1 + 4 * WTOP], F32, isOutput=True
        )

    with tile.TileContext(nc) as tc:
        with (
            tc.tile_pool(name="const", bufs=1) as cpool,
            tc.tile_pool(name="xt", bufs=2) as xtpool,
            tc.tile_pool(name="sc", bufs=2) as scpool,
            tc.tile_pool(name="work", bufs=1) as wpool,
            tc.tile_pool(name="ps_tp", bufs=2, space="PSUM") as ps_tp,
            tc.tile_pool(name="ps_m", bufs=2, space="PSUM") as ps_m,
            tc.tile_pool(name="ps_b", bufs=2, space="PSUM") as ps_b,
            tc.tile_pool(name="ps_2", bufs=1, space="PSUM") as ps_2,
        ):
            res_t = wpool.tile([128, NCH * CW], F16)
            nc.vector.memset(res_t[:, 0:NCH * CW:CW], 1.0)
            res2_t = wpool.tile([128, OVER * CW], F16)
            nc.vector.memset(res2_t[:, 0:OVER * CW:CW], 1.0)

            id_t = cpool.tile([128, 128], F32)
            nc.scalar.dma_start(id_t[:], id_d[:])
            id16_t = cpool.tile([128, 128], F16)
            nc.vector.tensor_copy(id16_t[:], id_t[:])

            qT_f32 = cpool.tile([128, KB * 32], F32)
            for k in range(KB):
                nc.scalar.dma_start(
                    qT_f32[:, 32 * k:32 * k + 32], qT_d[128 * k:128 * k + 128, :]
                )
            qT_t = cpool.tile([128, KB * 32], F16)
            nc.vector.tensor_copy(qT_t[:], qT_f32[:])

            iota_i = cpool.tile([32, 8 * MR], I32)
            nc.gpsimd.iota(iota_i[:], pattern=[[1, 8 * MR]], base=0,
                           channel_multiplier=0)
            iota_f = cpool.tile([32, 8 * MR], F32)
            nc.vector.tensor_copy(iota_f[:], iota_i[:])

            scratch_t = wpool.tile([128, 512], F32)
            cand_t = wpool.tile([128, 4 * WTOP], F32)
            p1src_t = wpool.tile([32, 8 * WTOP], F32)    # G0|G1 cands (192)
            p2src_t = wpool.tile([32, 128 + 4 * WTOP], F32)  # P1 | G2 (224)
            pfin_t = wpool.tile([32, 128], F32)          # P1 then P2
            g3_t = wpool.tile([32, 4 * WTOP], F32)
            c2_t = wpool.tile([32, 16 * WTOP], F32)
            tt_t = wpool.tile([32, 128], F32)
            ge_t = wpool.tile([32, 4 * WTOP], F32)
            mrg_t = wpool.tile([32, 80 + 4 * WTOP], F32)  # -P2[48:] | -G3f
            minb_t = wpool.tile([32, 8 * MR], F32)
            cnt_t = wpool.tile([32, 1], F32)
            eq_t = wpool.tile([32, 8 * MR], F32)
            thn_t = wpool.tile([32, 1], F32)
            thr4_t = wpool.tile([32, 4], F32)
            b_th = wpool.tile([128, 1], F32)
            rz_t = wpool.tile([32, 1], F32)
            o2_t = wpool.tile([32, H], F32)
            w16_t = wpool.tile([128, 2048], F16)
            wT_sb = [
                wpool.tile([128, 512], F16, name=f"wT_sb{u}") for u in range(4)
            ]
            sel_t = wpool.tile([128, 32], F32)
            nc.vector.memset(sel_t[:], 0.0)
            for g in range(4):
                nc.vector.tensor_copy(
                    sel_t[32 * g:32 * g + 32, :],
                    id_t[32 * g:32 * g + 32, 32 * g:32 * g + 32],
                )
            sba_t = wpool.tile([128, 385], F32)
            sbb_t = wpool.tile([128, 384], F32)

            cand4 = cand_t.rearrange("(j p) c -> p j c", j=4)

            def res_chunk(bb, c):
                """Residency slice [128, CW] for chunk c of batch bb."""
                if bb % 2 == 1 and c < OVER:
                    return res2_t[:, CW * c:CW * c + CW]
                return res_t[:, CW * c:CW * c + CW]

            def res_win(bb, w):
                """Residency slice [128, 4*CW] for window w of batch bb
                (windows never straddle the res/res2 boundary)."""
                c0 = 4 * w
                if bb % 2 == 1 and c0 < OVER:
                    return res2_t[:, CW * c0:CW * (c0 + 4)]
                return res_t[:, CW * c0:CW * (c0 + 4)]

            for b in range(BPC):
                row0 = b * N
                sc_t = scpool.tile([128, 2048], F32, name="scores")
                e16_t = scpool.tile([128, 2048], F16, name="expsc")

                def postproc(t, pw):
                    """Strip copy + exp + L1 top-24 for column group t,
                    then the hidden merge stage it unlocks."""
                    cs = slice(512 * t, 512 * t + 512)
                    nc.scalar.activation(sc_t[:, cs], pw[:], COPY)
                    nc.scalar.activation(
                        e16_t[:, cs], pw[:], EXP, scale=CSCALE,
                    )
                    cnd = cand_t[:, WTOP * t:WTOP * t + WTOP]
                    nc.vector.max(cnd[:, 0:8], sc_t[:, cs])
                    nc.vector.match_replace(
                        scratch_t[:], cnd[:, 0:8], sc_t[:, cs], NEG
                    )
                    nc.vector.max(cnd[:, 8:16], scratch_t[:])
                    nc.vector.match_replace(
                        scratch_t[:], cnd[:, 8:16], scratch_t[:], NEG
                    )
                    nc.vector.max(cnd[:, 16:24], scratch_t[:])
                    if THETA_SIMPLE:
                        return
                    if t == 1:
                        # P1 = top-128 of G0|G1's 192 candidates (hidden)
                        for jj in range(4):
                            nc.scalar.dma_start(
                                p1src_t[:, 48 * jj:48 * jj + 48],
                                cand_t[32 * jj:32 * jj + 32, 0:2 * WTOP],
                            )
                        for r in range(16):
                            nc.vector.max(
                                pfin_t[:, 8 * r:8 * r + 8], p1src_t[:]
                            )
                            if r < 15:
                                nc.vector.match_replace(
                                    p1src_t[:], pfin_t[:, 8 * r:8 * r + 8],
                                    p1src_t[:], NEG,
                                )
                    elif t == 2:
                        # P2 = top-128 of P1 | G2's 96 cands (hidden)
                        nc.vector.tensor_copy(p2src_t[:, 0:128], pfin_t[:])
                        for jj in range(4):
                            nc.scalar.dma_start(
                                p2src_t[:, 128 + WTOP * jj:
                                        128 + WTOP * jj + WTOP],
                                cand_t[32 * jj:32 * jj + 32,
                                       2 * WTOP:3 * WTOP],
                            )
                        for r in range(16):
                            nc.vector.max(
                                pfin_t[:, 8 * r:8 * r + 8], p2src_t[:]
                            )
                            if r < 15:
                                nc.vector.match_replace(
                                    p2src_t[:], pfin_t[:, 8 * r:8 * r + 8],
                                    p2src_t[:], NEG,
                                )

                # ---- Phase A: stream windows.
                pending = None
                pw = None
                for w in range(WPB):
                    jq = w % 4          # partition strip
                    nc.sync.dma_start(
                        res_win(b, w).rearrange(
                            "p (c e) -> p c e", c=4
                        )[:, :, 1:1 + H],
                        x_d[row0 + 512 * w:row0 + 512 * w + 512, :].rearrange(
                            "(c p) h -> p c h", p=128
                        ),
                    )
                    xt = xtpool.tile([128, KB * 512], F16)
                    xt3 = xt.rearrange("p (k i) -> p k i", k=KB)
                    nc.sync.dma_start(
                        xt3[:, 0:KT, :],
                        xt_d[:, row0 + 512 * w:row0 + 512 * w + 512].rearrange(
                            "(k p) n -> p k n", p=128
                        ),
                    )
                    if jq == 0:
                        pw = ps_m.tile([128, 512], F32, name="pw")
                    # mm1a: h-blocks 0..KT-1, N=512, straight off the DMA
                    for k in range(KT):
                        nc.tensor.matmul(
                            pw[32 * jq:32 * jq + 32, :],
                            qT_t[:, 32 * k:32 * k + 32],
                            xt[:, 512 * k:512 * k + 512],
                            start=(k == 0), stop=False,
                            skip_group_check=True,
                            tile_position=(0, 32 * jq),
                        )
                    for cw in range(4):
                        c = 4 * w + cw
                        src = res_chunk(b, c)[:, 1:1 + H]
                        # transpose h-blocks KT..5 of the fp16 chunk
                        tp = ps_tp.tile([128, KB - KT, 128], F16, name="tp")
                        for k in range(KT, KB):
                            nc.tensor.matmul(
                                tp[:, k - KT, :],
                                src[:, 128 * k:128 * k + 128],
                                id16_t[:],
                                is_transpose=True, start=True, stop=True,
                                skip_group_check=True,
                            )
                        dst = xt3[:, KT:KB, 128 * cw:128 * cw + 128]
                        nc.scalar.activation(dst, tp[:], COPY)
                        if cw % 2 == 1:
                            # mm1b: blocks KT..5 for this chunk pair, N=256
                            pcs = slice(128 * (cw - 1), 128 * (cw - 1) + 256)
                            for k in range(KT, KB):
                                nc.tensor.matmul(
                                    pw[32 * jq:32 * jq + 32, pcs],
                                    qT_t[:, 32 * k:32 * k + 32],
                                    xt[:, 512 * k + 128 * (cw - 1):
                                       512 * k + 128 * (cw - 1) + 256],
                                    start=False,
                                    stop=(k == KB - 1),
                                    skip_group_check=True,
                                    tile_position=(0, 32 * jq),
                                )
                    if pending is not None:
                        postproc(*pending)
                        pending = None
                    if jq == 3:
                        pending = (w // 4, pw)
                postproc(*pending)
                pending = None

                # ---- Phase A2 (exposed): theta = 128th of P2 | G3f via
                #      bottom-rank extraction.
                if THETA_SIMPLE:
                    for jj in range(4):
                        nc.scalar.dma_start(
                            c2_t[:, 96 * jj:96 * jj + 96],
                            cand_t[32 * jj:32 * jj + 32, :],
                        )
                    for r in range(16):
                        nc.vector.max(tt_t[:, 8 * r:8 * r + 8], c2_t[:])
                        if r < 15:
                            nc.vector.match_replace(
                                c2_t[:], tt_t[:, 8 * r:8 * r + 8],
                                c2_t[:], NEG,
                            )
                    for jj in range(4):
                        nc.scalar.dma_start(
                            b_th[32 * jj:32 * jj + 32, :], tt_t[:, 127:128]
                        )
                else:
                    for jj in range(4):
                        nc.scalar.dma_start(
                            g3_t[:, WTOP * jj:WTOP * jj + WTOP],
                            cand_t[32 * jj:32 * jj + 32, 3 * WTOP:4 * WTOP],
                        )
                    t0 = pfin_t[:, 127:128]
                    # ge = 1[g3 >= t0]; c' = row-sum(ge)
                    nc.vector.tensor_scalar(
                        ge_t[:], g3_t[:], t0, None, ALU.is_ge
                    )
                    nc.vector.tensor_reduce(cnt_t[:], ge_t[:], AXX, ALU.add)
                    # mrg = [-P2[48:128] | (-G3 masked to -BIG below t0)]
                    nc.vector.tensor_scalar_mul(
                        mrg_t[:, 0:80], pfin_t[:, 48:128], -1.0
                    )
                    nc.vector.tensor_scalar(
                        mrg_t[:, 80:176], g3_t[:], t0, -BIG, ALU.is_lt,
                        ALU.mult
                    )
                    nc.vector.tensor_tensor(
                        mrg_t[:, 80:176], mrg_t[:, 80:176], g3_t[:],
                        ALU.subtract
                    )
                    for r in range(MR):
                        nc.vector.max(minb_t[:, 8 * r:8 * r + 8], mrg_t[:])
                        if r < MR - 1:
                            nc.vector.match_replace(
                                mrg_t[:], minb_t[:, 8 * r:8 * r + 8],
                                mrg_t[:], NEG,
                            )
                    # theta = -minb[c']
                    nc.vector.tensor_scalar(
                        eq_t[:], iota_f[:], cnt_t[:, 0:1], None, ALU.is_equal
                    )
                    nc.vector.tensor_tensor(
                        eq_t[:], eq_t[:], minb_t[:], ALU.mult
                    )
                    nc.vector.tensor_reduce(thn_t[:], eq_t[:], AXX, ALU.add)
                    nc.vector.tensor_scalar_mul(thr4_t[:, 0:1], thn_t[:], -1.0)
                    for jj in range(4):
                        nc.scalar.dma_start(
                            b_th[32 * jj:32 * jj + 32, :], thr4_t[:, 0:1]
                        )

                if DEBUG_DUMP:
                    nc.scalar.dma_start(
                        dbg_d[128 * b:128 * b + 128, 0:2048], sc_t[:]
                    )
                    nc.scalar.dma_start(
                        dbg_d[128 * b:128 * b + 128, 2048:2049], b_th[:]
                    )
                    nc.scalar.dma_start(
                        dbg_d[128 * b:128 * b + 32, 2049:2049 + 4 * WTOP],
                        cand_t[0:32, :],
                    )

                # ---- Phase B interleaved with stt: per quarter u,
                #      w16 = 1[s>=theta]*E (fp16), then 4 fp16
                #      transposes of the quarter -> wT_sb[u].
                for u in range(4):
                    us = slice(512 * u, 512 * u + 512)
                    nc.vector.scalar_tensor_tensor(
                        w16_t[:, us], sc_t[:, us], b_th[:, 0:1],
                        e16_t[:, us],
                        ALU.is_ge, ALU.mult,
                    )
                    wtp = ps_b.tile([128, 512], F16, name="wtp")
                    for tt in range(4):
                        nc.tensor.matmul(
                            wtp[:, 128 * tt:128 * tt + 128],
                            w16_t[:, 512 * u + 128 * tt:
                                  512 * u + 128 * tt + 128],
                            id16_t[:],
                            is_transpose=True, start=True, stop=True,
                            skip_group_check=True,
                        )
                    nc.scalar.activation(wT_sb[u][:], wtp[:], COPY)

                # ---- Phase C: mm2 out = (w @ x) / Z, Z from ones column
                p2a = ps_2.tile([128, 385], F32)
                p2b = ps_2.tile([128, 384], F32)
                for i in range(NCH):
                    u = i // 16
                    tt = i % 4
                    jq = (i // 4) % 4
                    g = i % 4   # concurrent col-group (partition strip)
                    lhs = wT_sb[u][:, 128 * tt + 32 * jq:128 * tt + 32 * jq + 32]
                    src = res_chunk(b, i)
                    nc.tensor.matmul(
                        p2a[32 * g:32 * g + 32, :], lhs, src[:, 0:385],
                        start=(i < 4), stop=(i >= NCH - 4),
                        skip_group_check=True,
                        tile_position=(0, 32 * g),
                    )
                    nc.tensor.matmul(
                        p2b[32 * g:32 * g + 32, :], lhs, src[:, 385:CW],
                        start=(i < 4), stop=(i >= NCH - 4),
                        skip_group_check=True,
                        tile_position=(0, 32 * g),
                    )
                # merge the 4 strip partials: out[q] = sum_g partial[32g+q]
                nc.scalar.activation(sba_t[:], p2a[:], COPY)
                nc.scalar.activation(sbb_t[:], p2b[:], COPY)
                nc.tensor.matmul(
                    p2a[0:32, :], sel_t[:], sba_t[:],
                    start=True, stop=True, skip_group_check=True,
                )
                nc.tensor.matmul(
                    p2b[0:32, :], sel_t[:], sbb_t[:],
                    start=True, stop=True, skip_group_check=True,
                )
                nc.vector.reciprocal(rz_t[:], p2a[0:32, 0:1])
                nc.scalar.activation(
                    o2_t[:, 0:384], p2a[0:32, 1:385], COPY, scale=rz_t[:]
                )
                nc.scalar.activation(
                    o2_t[:, 384:768], p2b[0:32, :], COPY, scale=rz_t[:]
                )
                nc.scalar.dma_start(out_d[Q * b:Q * b + Q, :], o2_t[:])

    # Split multi-wait instructions to the TRN2 1-wait-per-instruction limit
    # (the standard Bacc.compile() passes, skipped on the bass2jax run path).
    import bass_rust as _bass_rust
    _bass_rust.move_matmul_waits_to_ldweights(nc.m)
    _bass_rust.generate_event_semaphores(nc)
    return nc


def _get_nc():
    global _built
    if _built is None:
        _built = _build()
    return _built


def run(inputs, trace=False):
    from concourse.bass_utils import run_bass_kernel_spmd

    x = np.asarray(inputs["input"], dtype=np.float32)
    seed = np.ascontiguousarray(np.asarray(inputs["seed"], dtype=np.float32))
    nc = _get_nc()
    seedT = np.ascontiguousarray(seed[0].T)
    ident = np.eye(128, dtype=np.float32)
    x16 = x.astype(np.float16).reshape(B * N, H)
    in_maps = []
    for c in range(NCORES):
        xb = np.ascontiguousarray(x16[BPC * N * c:BPC * N * (c + 1)])
        xbt = np.ascontiguousarray(xb[:, 0:KT * 128].T)
        in_maps.append(
            {"x16": xb, "x16t": xbt, "seedT": seedT, "ident": ident}
        )
    res = run_bass_kernel_spmd(nc, in_maps, list(range(NCORES)), trace=trace)
    out = np.empty((B, Q, H), np.float32)
    for c in range(NCORES):
        out[BPC * c:BPC * (c + 1)] = res.results[c]["out"].reshape(BPC, Q, H)
    return out, res


def kernel(**inputs):
    out, _ = run(inputs)
    return out
